# revision 23
# baseline (speedup 1.0000x reference)
"""CQAttention Trainium2 kernel.

Math (per batch b, H=256, q=2048, d=8192):
  Qp   = gelu(Q @ W.T + b)                       [q, H]
  S    = C @ Qp.T                                [d, q]
  P    = softmax(S, axis=q)
  out  = P @ Qp + C                              [d, H]

Sharding: data-parallel over batch, one batch per NeuronCore (8 cores).

Per-core pipeline (all matmuls contract over the feature dim or q):
  - Q^T, W^T via PE transposes; QpT = gelu(W Q^T + b) with per-partition bias
    on the ACT engine; Qp (natural, bf16) by transposing QpT back, augmented
    with a ones column so the softmax denominator falls out of the second
    matmul's PSUM accumulation.
  - Per 512-row chunk of C: transpose C tiles to put the feature dim on
    partitions; logits^T tiles [q=128, d=512] with fp16 operands (full PE
    rate, ~11-bit mantissa; bf16 fails the 2e-2 gate, fp32r is ~8% slower
    on the moving operand); exp on ACT straight from PSUM to bf16 (softmax
    without max-subtraction: |logits| < ~70 so fp32 exp is safe); attended
    accumulated over the 16 q-tiles into PSUM [d=128, 257] where column 256
    is the row-sum (ones column on Qp); fused epilogue
    out = (attended * 1/rowsum) + C in one DVE op per tile.
  - Startup: all DMAs issued up front (W, C0, bias, Q, C1-3); dummy PE
    transposes warm the PE clock ramp while transfers land; the whole
    Q-side setup (Q^T, linear+gelu, Qp) runs as a dedicated phase so ACT
    loads the Gelu table once and the Exp table once (interleaving them
    into chunk 0 reloads tables 8x and stalls the attended matmuls).
  - Chunk pipeline: C loads 4 chunks ahead, C transposes 2 chunks ahead,
    attended lags logits/exp by 2 q-tiles.
  - Last chunk runs attended dm-major with per-dm epilogue + output DMA so
    only 1/4 of the epilogue/store remains after the final PE instruction.
"""

from contextlib import ExitStack

import numpy as np

import concourse.mybir as mybir
import concourse.tile as tile
from concourse import bacc
from concourse.bass_utils import run_bass_kernel_spmd
from concourse.masks import make_identity

B, QL, D, H = 8, 2048, 8192, 256
N_CORES = 8
F32 = mybir.dt.float32
BF16 = mybir.dt.bfloat16
F16 = mybir.dt.float16

HC = H // 128      # feature chunks (2)
NQT = QL // 128    # q tiles (16)
DC = 512           # d-chunk size
NDC = D // DC      # d chunks (16)
NDM = DC // 128    # d tiles per chunk (4)

AF = mybir.ActivationFunctionType
ALU = mybir.AluOpType

LS = F16   # logits-matmul operand dtype


def build_body(ctx: ExitStack, tc: tile.TileContext, nc, Qd, Cd, Wd, bd, Od):
    singles = ctx.enter_context(tc.tile_pool(name="singles", bufs=1))
    qstat = ctx.enter_context(tc.tile_pool(name="qstat", bufs=1))
    cpool = ctx.enter_context(tc.tile_pool(name="cpool", bufs=5))
    ctpool = ctx.enter_context(tc.tile_pool(name="ctp", bufs=3))
    exppool = ctx.enter_context(tc.tile_pool(name="expp", bufs=2))
    outpool = ctx.enter_context(tc.tile_pool(name="outp", bufs=3))
    small = ctx.enter_context(tc.tile_pool(name="small", bufs=4))
    psum_l = ctx.enter_context(tc.tile_pool(name="psl", bufs=3, space="PSUM"))
    psum_t = ctx.enter_context(tc.tile_pool(name="pst", bufs=1, space="PSUM"))
    psum_a = ctx.enter_context(tc.tile_pool(name="psa", bufs=1, space="PSUM"))

    ident = singles.tile([128, 128], LS)
    make_identity(nc, ident)

    # --- Sync queue loads, critical-path order: W (gates the linear via
    # wt), tiny bias (gates the gelus), Q tiles 0-3 (gate the q chain),
    # C chunk 0 in halves, bulk q groups, C chunks 1-3. All f32 + DVE cast:
    # SWDGE casting DMAs measured ~78 GB/s and starve every concurrent
    # DMA of SDMA engines, so they are banned from the critical window. ---
    w_nat = singles.tile([128, HC, H], F32)
    nc.sync.dma_start(out=w_nat[:],
                      in_=Wd.rearrange("(a p) h -> p a h", p=128))
    w_src = singles.tile([128, HC, H], LS, name="w_bf")
    nc.vector.tensor_copy(w_src[:], w_nat[:])
    bias = singles.tile([128, HC, 1], F32)
    nc.sync.dma_start(out=bias[:, :, 0], in_=bd.rearrange("(c p) -> p c", p=128))

    q_nat = cpool.tile([128, NQT, H], F32, tag="qnat", bufs=1)
    q_src = cpool.tile([128, NQT, H], LS, tag="qbf", bufs=1)
    q_view = Qd.rearrange("(a p) h -> p a h", p=128)
    for qi in range(4):
        nc.sync.dma_start(out=q_nat[:, qi:qi + 1, :],
                          in_=q_view[:, qi:qi + 1, :])
        nc.vector.tensor_copy(q_src[:, qi:qi + 1, :],
                              q_nat[:, qi:qi + 1, :])

    c_nats = {}
    c_bfs = {}

    def c_load_setup(dc):
        c_nats[dc] = cpool.tile([128, NDM, H], F32, tag="cnat", bufs=4,
                                name=f"cnat{dc}")
        nc.sync.dma_start(
            out=c_nats[dc][:],
            in_=Cd[dc * DC:(dc + 1) * DC, :].rearrange("(a p) h -> p a h", p=128))

    def c_cast(dc):
        c_bf = cpool.tile([128, NDM, H], LS, tag="cbf", name=f"cbf{dc}")
        nc.vector.tensor_copy(c_bf[:], c_nats[dc][:])
        c_bfs[dc] = c_bf
        del c_nats[dc]

    # chunk 0 in halves so its cast/transposes start while the rest lands
    c_nats[0] = cpool.tile([128, NDM, H], F32, tag="cnat", bufs=4,
                           name="cnat0")
    c_bf0 = cpool.tile([128, NDM, H], LS, tag="cbf", name="cbf0")
    c0_view = Cd[0:DC, :].rearrange("(a p) h -> p a h", p=128)
    for half in range(2):
        sl = slice(2 * half, 2 * half + 2)
        nc.sync.dma_start(out=c_nats[0][:, sl, :], in_=c0_view[:, sl, :])
        nc.vector.tensor_copy(c_bf0[:, sl, :], c_nats[0][:, sl, :])
    c_bfs[0] = c_bf0
    del c_nats[0]

    for qg in range(1, 4):
        nc.sync.dma_start(out=q_nat[:, qg * 4:(qg + 1) * 4, :],
                          in_=q_view[:, qg * 4:(qg + 1) * 4, :])
    for _dc in range(1, 4):
        c_load_setup(_dc)

    def q_cast(qg):
        nc.vector.tensor_copy(q_src[:, qg * 4:(qg + 1) * 4, :],
                              q_nat[:, qg * 4:(qg + 1) * 4, :])

    # --- in-loop C prefetch: plain f32 SWDGE load on the idle GpSimd
    # queue; the DVE cast is issued at the END of the chunk body so it
    # never sits ahead of the latency-critical epilogue in the DVE queue.
    def c_load(dc):
        c_nats[dc] = cpool.tile([128, NDM, H], F32, tag="cnat", bufs=4,
                                name=f"cnat{dc}")
        nc.gpsimd.dma_start(
            out=c_nats[dc][:],
            in_=Cd[dc * DC:(dc + 1) * DC, :].rearrange("(a p) h -> p a h", p=128))

    # Setup-phase transposes rotate over the (idle) attended psum banks plus
    # the pt ring so the PE never stalls on a psum WAR against the DVE copy.
    _setup_pt = [0]

    def setup_pt_tile(name):
        k = _setup_pt[0] % 5
        _setup_pt[0] += 1
        if k < 4:
            return psum_a.tile([128, 512], LS, tag=f"a{k}", name=name)
        return psum_t.tile([128, 512], LS, tag="pt", name=name)

    # --- keep the PE continuously busy through the DMA-gated setup ---
    # HAM un-throttles the PE clock (1.2 -> 2.4 GHz) only after ~3.4us of
    # sustained busy; rotating warm transposes across 5 psum banks keeps
    # them dense (a single-bank ring WAW-serializes at ~290ns/op and the
    # window never fills).
    _warm_n = [0]

    def warm(n):
        for _ in range(n):
            k = _warm_n[0]
            _warm_n[0] += 1
            pt = setup_pt_tile(f"warm{k}")
            nc.tensor.transpose(pt[:, 0:128], ident[:], ident[:])

    warm(14)

    # --- preload the Gelu activation table before it gates the pipeline;
    # the Exp table load is triggered by the first exp right after the last
    # gelu (gelu and exp live in different table sets, so an early exp
    # preload just forces an extra ~1.3us reload of each). ---
    warm_act = small.tile([1, 1], F32, name="warm_act")
    nc.scalar.activation(warm_act[:], ident[0:1, 0:1], AF.Gelu)

    wt = qstat.tile([128, HC, H], LS)  # [h in-chunk, hc, o]

    def w_prep():
        for om in range(HC):
            for hc in range(HC):
                pt = setup_pt_tile(f"ptw{om}_{hc}")
                nc.tensor.transpose(pt[:, 0:128],
                                    w_src[:, om, hc * 128:(hc + 1) * 128], ident[:])
                nc.vector.tensor_copy(wt[:, hc, om * 128:(om + 1) * 128], pt[:, 0:128])

    # --- C transposes: ct [h part, hc, d] f16 ---
    # Chunk 0 transposes on the PE (they hide in the setup DMA-wait); all
    # later chunks go through the DMA xbar transpose (per-dm [128, 256] ->
    # [128, hc, 128] blocks), freeing ~0.9us of PE per chunk.
    def c_transpose(dc, c_bf, hc, pt_alloc):
        pt = pt_alloc(f"ptc{dc}_{hc}")
        for dm in range(NDM):
            nc.tensor.transpose(
                pt[:, dm * 128:(dm + 1) * 128],
                c_bf[:, dm, hc * 128:(hc + 1) * 128], ident[:])
        return pt

    cts = {}

    def c_prep(dc, pt_alloc):
        cts[dc] = ctpool.tile([128, HC, DC], LS, tag="ct", name=f"ct{dc}")
        for hc in range(HC):
            pt = c_transpose(dc, c_bfs[dc], hc, pt_alloc)
            nc.vector.tensor_copy(cts[dc][:, hc, :], pt[:])

    def c_prep_dma(dc):
        cts[dc] = ctpool.tile([128, HC, DC], LS, tag="ct", name=f"ct{dc}")
        c_bf = c_bfs[dc]
        for dm in range(NDM):
            nc.sync.dma_start(
                out=cts[dc][:, :, dm * 128:(dm + 1) * 128],
                in_=c_bf[:, dm, :], transpose=True)

    # --- Q-phase: Q^T -> linear+gelu; Qp natural via DMA xbar transpose ---
    qt = qstat.tile([128, HC, QL], LS)
    qpt = qstat.tile([128, HC, QL], LS)
    qp_f16 = qstat.tile([128, NQT, H], LS)
    qp = qstat.tile([128, NQT, H + 1], BF16)

    def q_group_tr(qg):
        # Q^T for this group of 4 q-tiles
        for hc in range(HC):
            pt = setup_pt_tile(f"ptq{qg}_{hc}")
            for k in range(4):
                qi = qg * 4 + k
                nc.tensor.transpose(
                    pt[:, k * 128:(k + 1) * 128],
                    q_src[:, qi, hc * 128:(hc + 1) * 128], ident[:])
            nc.vector.tensor_copy(qt[:, hc, qg * 512:(qg + 1) * 512], pt[:])

    def q_group_lin(qg):
        for om in range(HC):
            pl = psum_l.tile([128, 512], F32, tag="pl", name=f"plin{qg}_{om}")
            for hc in range(HC):
                nc.tensor.matmul(
                    pl[:],
                    wt[:, hc, om * 128:(om + 1) * 128],
                    qt[:, hc, qg * 512:(qg + 1) * 512],
                    start=(hc == 0),
                    stop=(hc == HC - 1),
                )
            nc.scalar.activation(
                qpt[:, om, qg * 512:(qg + 1) * 512], pl[:], AF.Gelu,
                bias=bias[:, om, :], scale=1.0,
            )

    def qp_dma(qg):
        # Qp natural for this group via the DMA xbar (no PE involvement):
        # qpt [h, q-block 512] -> qp_f16 [q, 4 tiles, h-block], then one DVE
        # cast f16 -> bf16 into qp (which carries the extra ones column).
        for hc in range(HC):
            nc.sync.dma_start(
                out=qp_f16[:, qg * 4:(qg + 1) * 4, hc * 128:(hc + 1) * 128],
                in_=qpt[:, hc, qg * 512:(qg + 1) * 512], transpose=True)
        nc.vector.tensor_copy(qp[:, qg * 4:(qg + 1) * 4, 0:H],
                              qp_f16[:, qg * 4:(qg + 1) * 4, :])

    def q_group_b0():
        # qp natural for group 0 on the PE: it is needed by the very first
        # attends, before the xbar path could deliver it
        for om in range(HC):
            pt = setup_pt_tile(f"ptp0_{om}")
            for k in range(4):
                nc.tensor.transpose(pt[:, k * 128:(k + 1) * 128],
                                    qpt[:, om, k * 128:(k + 1) * 128], ident[:])
            nc.vector.tensor_copy(
                qp[:, 0:4, om * 128:(om + 1) * 128],
                pt[:, 0:512].rearrange("p (a b) -> p a b", a=4))

    nc.vector.memset(qp[:, :, H:H + 1], 1.0)
    w_prep()
    q_group_tr(0)
    c_prep(0, setup_pt_tile)
    q_group_lin(0)
    q_cast(1)
    q_group_tr(1)
    q_group_lin(1)
    c_cast(1)
    c_prep_dma(1)
    q_cast(2)
    q_group_tr(2)
    q_group_lin(2)
    q_group_b0()
    c_cast(2)
    q_cast(3)
    q_group_tr(3)
    q_group_lin(3)
    for qg in range(1, 4):
        qp_dma(qg)
    c_cast(3)
    c_prep_dma(2)

    # Lag the attended matmuls three q-tiles behind logits+exp so the PE
    # never waits on the ACT exp latency, and so a new chunk's first pa
    # writes land after the previous chunk's epilogue reads free the
    # psum_a banks (LAG=2 measured ~250ns PE stalls at every chunk seam).
    LAG = 3
    for dc in range(NDC):
        c_bf = c_bfs[dc]
        ct = cts[dc]
        expt = exppool.tile([128, NQT, DC], BF16)
        pa = [psum_a.tile([128, H + 1], F32, tag=f"a{dm}", name=f"pa{dm}")
              for dm in range(NDM)]
        nxt = dc + 2
        last = dc == NDC - 1

        def logits_exp(qi):
            pl = psum_l.tile([128, DC], F32, tag="pl")
            for hc in range(HC):
                nc.tensor.matmul(
                    pl[:],
                    qpt[:, hc, qi * 128:(qi + 1) * 128],
                    ct[:, hc, :],
                    start=(hc == 0),
                    stop=(hc == HC - 1),
                )
            nc.scalar.activation(expt[:, qi, :], pl[:], AF.Exp)

        def attend(qj, dms):
            for dm in dms:
                nc.tensor.matmul(
                    pa[dm][:],
                    expt[:, qj, dm * 128:(dm + 1) * 128],
                    qp[:, qj, :],
                    start=(qj == 0),
                    stop=(qj == NQT - 1),
                )

        def epilogue(dm, o_sb):
            rec = small.tile([128, 1], F32)
            nc.vector.reciprocal(rec[:], pa[dm][:, H:H + 1])
            # C addend in f16: adds ~4e-4 absmax error, well under the gate
            nc.vector.scalar_tensor_tensor(
                o_sb[:, dm, :], pa[dm][:, 0:H], rec[:], c_bf[:, dm, :],
                ALU.mult, ALU.add,
            )

        if not last:
            for step in range(NQT + LAG):
                if step == 2 and dc + 4 < NDC:
                    c_load(dc + 4)
                if step == 6 and nxt < NDC and nxt not in cts:
                    c_prep_dma(nxt)
                if step < NQT:
                    logits_exp(step)
                if step >= LAG:
                    attend(step - LAG, range(NDM))

            o_sb = outpool.tile([128, NDM, H], F32)
            for dm in range(NDM):
                epilogue(dm, o_sb)
            nc.sync.dma_start(
                out=Od[dc * DC:(dc + 1) * DC, :].rearrange(
                    "(a p) h -> p a h", p=128),
                in_=o_sb[:])
            if dc + 4 < NDC:
                # cast AFTER the epilogue in DVE program order: the load
                # landed long ago, and the epilogue must not queue behind it
                c_cast(dc + 4)
        else:
            # dm-major: interleave dm0's attended with the logits/exp steps,
            # then sweep dm1-3; epilogue + store per dm so the post-PE tail
            # is only dm3's epilogue + 128-row store.
            o_sb = outpool.tile([128, NDM, H], F32)
            for step in range(NQT):
                logits_exp(step)
                if step >= LAG:
                    attend(step - LAG, [0])
            for k in range(NQT - LAG, NQT):
                attend(k, [0])
            for dm in range(1, NDM):
                for qj in range(NQT):
                    attend(qj, [dm])
            for dm in range(NDM):
                epilogue(dm, o_sb)
                nc.sync.dma_start(
                    out=Od[dc * DC + dm * 128:dc * DC + (dm + 1) * 128, :]
                        .rearrange("(a p) h -> p a h", p=128),
                    in_=o_sb[:, dm:dm + 1, :])
        del c_bfs[dc], cts[dc]


def build_nc():
    nc = bacc.Bacc("TRN2", target_bir_lowering=False, debug=False,
                   num_devices=N_CORES)
    Qd = nc.dram_tensor("Q", [QL, H], F32, kind="ExternalInput")
    Cd = nc.dram_tensor("C", [D, H], F32, kind="ExternalInput")
    Wd = nc.dram_tensor("W", [H, H], F32, kind="ExternalInput")
    bd = nc.dram_tensor("b", [H], F32, kind="ExternalInput")
    Od = nc.dram_tensor("out", [D, H], F32, kind="ExternalOutput")
    with tile.TileContext(nc) as tc:
        with ExitStack() as ctx:
            build_body(ctx, tc, nc, Qd[:], Cd[:], Wd[:], bd[:], Od[:])
    nc.finalize()
    return nc


_NC = None


def get_nc():
    global _NC
    if _NC is None:
        _NC = build_nc()
    return _NC


def kernel(Q, C, W, b):
    assert Q.shape == (B, QL, H) and C.shape == (B, D, H)
    nc = get_nc()
    in_maps = [
        {
            "Q": np.ascontiguousarray(Q[i], dtype=np.float32),
            "C": np.ascontiguousarray(C[i], dtype=np.float32),
            "W": np.ascontiguousarray(W, dtype=np.float32),
            "b": np.ascontiguousarray(b, dtype=np.float32),
        }
        for i in range(N_CORES)
    ]
    res = run_bass_kernel_spmd(nc, in_maps, core_ids=list(range(N_CORES)))
    return np.stack([res.results[i]["out"] for i in range(N_CORES)], axis=0)



# revision 24
# speedup vs baseline: 1.0498x; 1.0498x over previous
"""CQAttention Trainium2 kernel.

Math (per batch b, H=256, q=2048, d=8192):
  Qp   = gelu(Q @ W.T + b)                       [q, H]
  S    = C @ Qp.T                                [d, q]
  P    = softmax(S, axis=q)
  out  = P @ Qp + C                              [d, H]

Sharding: data-parallel over batch, one batch per NeuronCore (8 cores).

Per-core pipeline (all matmuls contract over the feature dim or q):
  - Q^T, W^T via PE transposes; QpT = gelu(W Q^T + b) with per-partition bias
    on the ACT engine; Qp (natural, bf16) by transposing QpT back, augmented
    with a ones column so the softmax denominator falls out of the second
    matmul's PSUM accumulation.
  - Per 512-row chunk of C: C^T tiles via the DMA xbar transpose (chunks 1+;
    chunk 0 on the PE where it hides in the setup DMA-wait); logits^T tiles
    [q=128, d=512] with fp16 operands (full PE rate, ~11-bit mantissa; bf16
    fails the 2e-2 gate); exp on ACT straight from PSUM to bf16 (softmax
    without max-subtraction: |logits| < ~70 so fp32 exp is safe); attended
    accumulated over the 16 q-tiles into PSUM [d=128, 257] where column 256
    is the row-sum (ones column on Qp); fused epilogue
    out = (attended * 1/rowsum) + C in one DVE op per tile (C addend in f16:
    ~4e-4 absmax error, far under the gate).
  - Startup: all DMAs issued up front; dense dummy PE transposes rotating
    over 5 psum banks warm the HAM clock gate (1.2 -> 2.4 GHz needs ~3.4us
    of sustained PE busy); the whole Q-side setup runs as a dedicated phase
    so ACT loads the Gelu table once, then the Exp table once (warm_exp is
    data-pinned after the last gelu so the scheduler cannot hoist it and
    force a second pair of ~1.3us table reloads).
  - Chunk pipeline: C loads 4 chunks ahead as plain f32 SWDGE DMAs on the
    idle GpSimd queue (SWDGE *casting* DMAs run ~78 GB/s and starve all
    concurrent DMA, so casts stay on the DVE); each chunk's f32->f16 cast
    is issued at the END of a chunk body so it never sits ahead of the
    latency-critical epilogue in the DVE queue; C^T 2 chunks ahead;
    attended lags logits/exp by 3 q-tiles (LAG=2 left ~250ns PE stalls at
    every chunk seam against the epilogue's psum_a reads).
  - Last chunk runs attended dm-major with per-dm epilogue + output DMA so
    only 1/4 of the epilogue/store remains after the final PE instruction.
"""

from contextlib import ExitStack

import numpy as np

import concourse.mybir as mybir
import concourse.tile as tile
from concourse import bacc
from concourse.bass_utils import run_bass_kernel_spmd
from concourse.masks import make_identity

B, QL, D, H = 8, 2048, 8192, 256
N_CORES = 8
F32 = mybir.dt.float32
BF16 = mybir.dt.bfloat16
F16 = mybir.dt.float16

HC = H // 128      # feature chunks (2)
NQT = QL // 128    # q tiles (16)
DC = 512           # d-chunk size
NDC = D // DC      # d chunks (16)
NDM = DC // 128    # d tiles per chunk (4)

AF = mybir.ActivationFunctionType
ALU = mybir.AluOpType

LS = F16   # logits-matmul operand dtype


def build_body(ctx: ExitStack, tc: tile.TileContext, nc, Qd, Cd, Wd, bd, Od):
    singles = ctx.enter_context(tc.tile_pool(name="singles", bufs=1))
    qstat = ctx.enter_context(tc.tile_pool(name="qstat", bufs=1))
    cpool = ctx.enter_context(tc.tile_pool(name="cpool", bufs=5))
    ctpool = ctx.enter_context(tc.tile_pool(name="ctp", bufs=3))
    exppool = ctx.enter_context(tc.tile_pool(name="expp", bufs=2))
    outpool = ctx.enter_context(tc.tile_pool(name="outp", bufs=3))
    small = ctx.enter_context(tc.tile_pool(name="small", bufs=4))
    psum_l = ctx.enter_context(tc.tile_pool(name="psl", bufs=3, space="PSUM"))
    psum_t = ctx.enter_context(tc.tile_pool(name="pst", bufs=1, space="PSUM"))
    psum_a = ctx.enter_context(tc.tile_pool(name="psa", bufs=1, space="PSUM"))

    ident = singles.tile([128, 128], LS)
    make_identity(nc, ident)

    # --- issue all startup DMAs, critical-path order ---
    w_nat = singles.tile([128, HC, H], F32)  # [o in-chunk, om, h]

    c_nats = {}
    c_bfs = {}

    def c_load(dc):
        # In-loop prefetch: plain f32 SWDGE DMA on the idle GpSimd queue.
        c_nats[dc] = cpool.tile([128, NDM, H], F32, tag="cnat", bufs=4,
                                name=f"cnat{dc}")
        nc.gpsimd.dma_start(
            out=c_nats[dc][:],
            in_=Cd[dc * DC:(dc + 1) * DC, :].rearrange("(a p) h -> p a h", p=128))

    def c_cast(dc):
        c_bf = cpool.tile([128, NDM, H], LS, tag="cbf", name=f"cbf{dc}")
        nc.vector.tensor_copy(c_bf[:], c_nats[dc][:])
        c_bfs[dc] = c_bf
        del c_nats[dc]

    # chunk 0 and q group 0 split into 128-row pieces across DMA queues,
    # triggers interleaved so both streams land early and alternately
    c_nats[0] = cpool.tile([128, NDM, H], F32, tag="cnat", bufs=4,
                           name="cnat0")
    c_bf0 = cpool.tile([128, NDM, H], LS, tag="cbf", name="cbf0")
    c0_view = Cd[0:DC, :].rearrange("(a p) h -> p a h", p=128)
    q_nat = cpool.tile([128, NQT, H], F32, tag="qnat", bufs=1)
    q_src = cpool.tile([128, NQT, H], LS, tag="qbf", bufs=1)
    q_view = Qd.rearrange("(a p) h -> p a h", p=128)
    for k in range(NDM):
        sl = slice(k, k + 1)
        nc.sync.dma_start(out=c_nats[0][:, sl, :], in_=c0_view[:, sl, :])
        nc.vector.tensor_copy(c_bf0[:, sl, :], c_nats[0][:, sl, :])
    c_bfs[0] = c_bf0
    del c_nats[0]

    # q group 0 pieces outrank w/bias: the last piece gates the first linear
    for qi in range(2):
        nc.sync.dma_start(out=q_nat[:, qi:qi + 1, :],
                          in_=q_view[:, qi:qi + 1, :])
        nc.vector.tensor_copy(q_src[:, qi:qi + 1, :],
                              q_nat[:, qi:qi + 1, :])
    nc.sync.dma_start(out=w_nat[:],
                      in_=Wd.rearrange("(a p) h -> p a h", p=128))
    w_src = singles.tile([128, HC, H], LS, name="w_bf")
    for om in range(HC):
        nc.vector.tensor_copy(w_src[:, om, :], w_nat[:, om, :])
    for qi in range(2, 4):
        nc.sync.dma_start(out=q_nat[:, qi:qi + 1, :],
                          in_=q_view[:, qi:qi + 1, :])
        nc.vector.tensor_copy(q_src[:, qi:qi + 1, :],
                              q_nat[:, qi:qi + 1, :])

    bias = singles.tile([128, HC, 1], F32)
    nc.sync.dma_start(out=bias[:, :, 0], in_=bd.rearrange("(c p) -> p c", p=128))
    # issue the remaining transfers now, but defer their DVE casts so the
    # latency-critical ct0/wt/qt copies aren't stuck behind bulk conversions
    for qg in range(1, 4):
        nc.sync.dma_start(out=q_nat[:, qg * 4:(qg + 1) * 4, :],
                          in_=q_view[:, qg * 4:(qg + 1) * 4, :])

    def q_cast(qg):
        nc.vector.tensor_copy(q_src[:, qg * 4:(qg + 1) * 4, :],
                              q_nat[:, qg * 4:(qg + 1) * 4, :])

    for _dc in range(1, 4):
        c_nats[_dc] = cpool.tile([128, NDM, H], F32, tag="cnat", bufs=4,
                                 name=f"cnat{_dc}")
        nc.sync.dma_start(
            out=c_nats[_dc][:],
            in_=Cd[_dc * DC:(_dc + 1) * DC, :].rearrange(
                "(a p) h -> p a h", p=128))

    # Setup-phase transposes rotate over the (idle) attended psum banks plus
    # the pt ring so the PE never stalls on a psum WAR against the DVE copy.
    _setup_pt = [0]

    def setup_pt_tile(name):
        k = _setup_pt[0] % 5
        _setup_pt[0] += 1
        if k < 4:
            return psum_a.tile([128, 512], LS, tag=f"a{k}", name=name)
        return psum_t.tile([128, 512], LS, tag="pt", name=name)

    # --- keep the PE continuously busy through the DMA-gated setup ---
    # HAM un-throttles the PE clock (1.2 -> 2.4 GHz) only after ~3.4us of
    # sustained busy; rotating the dummies over 5 psum banks keeps them
    # dense at ~107ns each (a single-bank ring WAW-serializes at ~290ns and
    # the activity window never fills, leaving the whole setup at 1.2 GHz).
    _warm_n = [0]

    def warm(n):
        for _ in range(n):
            k = _warm_n[0]
            _warm_n[0] += 1
            pt = setup_pt_tile(f"warm{k}")
            nc.tensor.transpose(pt[:, 0:128], ident[:], ident[:])

    warm(30)

    # --- preload the Gelu activation table before it gates the pipeline ---
    warm_act = small.tile([1, 1], F32, name="warm_act")
    nc.scalar.activation(warm_act[:], ident[0:1, 0:1], AF.Gelu)

    wt = qstat.tile([128, HC, H], LS)  # [h in-chunk, hc, o]

    def w_prep():
        for om in range(HC):
            for hc in range(HC):
                pt = setup_pt_tile(f"ptw{om}_{hc}")
                nc.tensor.transpose(pt[:, 0:128],
                                    w_src[:, om, hc * 128:(hc + 1) * 128], ident[:])
                nc.vector.tensor_copy(wt[:, hc, om * 128:(om + 1) * 128], pt[:, 0:128])

    # --- C transposes: ct [h part, hc, d] f16 ---
    # Chunk 0 transposes on the PE (they hide in the setup DMA-wait); all
    # later chunks go through the DMA xbar transpose (per-dm [128, 256] ->
    # [128, hc, 128] blocks), freeing ~0.9us of PE per chunk.
    def c_transpose(dc, c_bf, hc, pt_alloc):
        pt = pt_alloc(f"ptc{dc}_{hc}")
        for dm in range(NDM):
            nc.tensor.transpose(
                pt[:, dm * 128:(dm + 1) * 128],
                c_bf[:, dm, hc * 128:(hc + 1) * 128], ident[:])
        return pt

    def loop_pt_tile(name):
        return psum_t.tile([128, 512], LS, tag="pt", name=name)

    cts = {}

    def c_prep(dc, pt_alloc):
        cts[dc] = ctpool.tile([128, HC, DC], LS, tag="ct", name=f"ct{dc}")
        for hc in range(HC):
            pt = c_transpose(dc, c_bfs[dc], hc, pt_alloc)
            nc.vector.tensor_copy(cts[dc][:, hc, :], pt[:])

    def c_prep_dma(dc):
        cts[dc] = ctpool.tile([128, HC, DC], LS, tag="ct", name=f"ct{dc}")
        c_bf = c_bfs[dc]
        for dm in range(NDM):
            nc.sync.dma_start(
                out=cts[dc][:, :, dm * 128:(dm + 1) * 128],
                in_=c_bf[:, dm, :], transpose=True)

    # --- Q-phase: Q^T -> linear+gelu; Qp transposes overlap chunk 0 ---
    qt = qstat.tile([128, HC, QL], LS)
    qpt = qstat.tile([128, HC, QL], LS)
    qp = qstat.tile([128, NQT, H + 1], BF16)

    def q_group_a(qg):
        # Q^T + linear + gelu for this group of 4 q-tiles
        for hc in range(HC):
            pt = setup_pt_tile(f"ptq{qg}_{hc}")
            for k in range(4):
                qi = qg * 4 + k
                nc.tensor.transpose(
                    pt[:, k * 128:(k + 1) * 128],
                    q_src[:, qi, hc * 128:(hc + 1) * 128], ident[:])
            nc.vector.tensor_copy(qt[:, hc, qg * 512:(qg + 1) * 512], pt[:])
        for om in range(HC):
            pl = psum_l.tile([128, 512], F32, tag="pl", name=f"plin{qg}_{om}")
            for hc in range(HC):
                nc.tensor.matmul(
                    pl[:],
                    wt[:, hc, om * 128:(om + 1) * 128],
                    qt[:, hc, qg * 512:(qg + 1) * 512],
                    start=(hc == 0),
                    stop=(hc == HC - 1),
                )
            nc.scalar.activation(
                qpt[:, om, qg * 512:(qg + 1) * 512], pl[:], AF.Gelu,
                bias=bias[:, om, :], scale=1.0,
            )

    def q_group_b(qg, pt_alloc):
        # Qp natural for this group (PE + DVE only, no ACT)
        for om in range(HC):
            pt = pt_alloc(f"ptp{qg}_{om}")
            for k in range(4):
                qi = qg * 4 + k
                nc.tensor.transpose(pt[:, k * 128:(k + 1) * 128],
                                    qpt[:, om, qi * 128:(qi + 1) * 128], ident[:])
            nc.vector.tensor_copy(
                qp[:, qg * 4:(qg + 1) * 4, om * 128:(om + 1) * 128],
                pt[:, 0:512].rearrange("p (a b) -> p a b", a=4))

    c_prep(0, setup_pt_tile)
    w_prep()
    q_group_a(0)
    q_cast(1)
    q_group_a(1)
    c_cast(1)
    q_group_b(0, setup_pt_tile)
    c_prep_dma(1)
    q_cast(2)
    q_group_a(2)
    c_cast(2)
    q_cast(3)
    q_group_a(3)
    c_prep_dma(2)
    # PE filler between the last gelu and chunk 0's first exps so the ACT
    # engine catches up before the exp chain becomes the critical path
    q_group_b(1, setup_pt_tile)
    q_group_b(2, setup_pt_tile)
    c_cast(3)
    # preload the Exp table; the input is data-pinned to the last gelu's
    # output so the scheduler cannot hoist the table load ahead of the gelus
    # (gelu/exp live in different ACT table sets - hoisting forces a second
    # pair of ~1.3us reloads)
    warm_exp = small.tile([1, 1], F32, name="warm_exp")
    nc.scalar.activation(warm_exp[:], qpt[0:1, HC - 1, QL - 1:QL], AF.Exp)
    nc.vector.memset(qp[:, :, H:H + 1], 1.0)

    # Lag the attended matmuls three q-tiles behind logits+exp so the PE
    # never waits on the ACT exp latency, and so a new chunk's first pa
    # writes land after the previous chunk's epilogue reads free the
    # psum_a banks (LAG=2 measured ~250ns PE stalls at every chunk seam).
    LAG = 3
    for dc in range(NDC):
        c_bf = c_bfs[dc]
        ct = cts[dc]
        expt = exppool.tile([128, NQT, DC], BF16)
        pa = [psum_a.tile([128, H + 1], F32, tag=f"a{dm}", name=f"pa{dm}")
              for dm in range(NDM)]
        nxt = dc + 2
        last = dc == NDC - 1

        def logits_exp(qi):
            pl = psum_l.tile([128, DC], F32, tag="pl")
            for hc in range(HC):
                nc.tensor.matmul(
                    pl[:],
                    qpt[:, hc, qi * 128:(qi + 1) * 128],
                    ct[:, hc, :],
                    start=(hc == 0),
                    stop=(hc == HC - 1),
                )
            nc.scalar.activation(expt[:, qi, :], pl[:], AF.Exp)

        def attend(qj, dms):
            for dm in dms:
                nc.tensor.matmul(
                    pa[dm][:],
                    expt[:, qj, dm * 128:(dm + 1) * 128],
                    qp[:, qj, :],
                    start=(qj == 0),
                    stop=(qj == NQT - 1),
                )

        def epilogue(dm, o_sb):
            rec = small.tile([128, 1], F32)
            nc.vector.reciprocal(rec[:], pa[dm][:, H:H + 1])
            # C addend in f16: adds ~4e-4 absmax error, well under the gate
            nc.vector.scalar_tensor_tensor(
                o_sb[:, dm, :], pa[dm][:, 0:H], rec[:], c_bf[:, dm, :],
                ALU.mult, ALU.add,
            )

        if not last:
            for step in range(NQT + LAG):
                if dc == 0 and step == 9:
                    q_group_b(3, loop_pt_tile)
                if step == 2 and dc + 4 < NDC:
                    c_load(dc + 4)
                if step == 6 and nxt < NDC and nxt not in cts:
                    c_prep_dma(nxt)
                if step < NQT:
                    logits_exp(step)
                if step >= LAG:
                    attend(step - LAG, range(NDM))

            o_sb = outpool.tile([128, NDM, H], F32)
            for dm in range(NDM):
                epilogue(dm, o_sb)
            nc.sync.dma_start(
                out=Od[dc * DC:(dc + 1) * DC, :].rearrange(
                    "(a p) h -> p a h", p=128),
                in_=o_sb[:])
            if dc + 4 < NDC:
                # cast AFTER the epilogue in DVE program order: the load
                # landed long ago, and the epilogue must not queue behind it
                c_cast(dc + 4)
        else:
            # dm-major: interleave dm0's attended with the logits/exp steps,
            # then sweep dm1-3; epilogue + store per dm so the post-PE tail
            # is only dm3's epilogue + 128-row store.
            o_sb = outpool.tile([128, NDM, H], F32)
            for step in range(NQT):
                logits_exp(step)
                if step >= LAG:
                    attend(step - LAG, [0])
            for k in range(NQT - LAG, NQT):
                attend(k, [0])
            for dm in range(1, NDM):
                for qj in range(NQT):
                    attend(qj, [dm])
            for dm in range(NDM):
                epilogue(dm, o_sb)
                nc.sync.dma_start(
                    out=Od[dc * DC + dm * 128:dc * DC + (dm + 1) * 128, :]
                        .rearrange("(a p) h -> p a h", p=128),
                    in_=o_sb[:, dm:dm + 1, :])
        del c_bfs[dc], cts[dc]


def build_nc():
    nc = bacc.Bacc("TRN2", target_bir_lowering=False, debug=False,
                   num_devices=N_CORES)
    Qd = nc.dram_tensor("Q", [QL, H], F32, kind="ExternalInput")
    Cd = nc.dram_tensor("C", [D, H], F32, kind="ExternalInput")
    Wd = nc.dram_tensor("W", [H, H], F32, kind="ExternalInput")
    bd = nc.dram_tensor("b", [H], F32, kind="ExternalInput")
    Od = nc.dram_tensor("out", [D, H], F32, kind="ExternalOutput")
    with tile.TileContext(nc) as tc:
        with ExitStack() as ctx:
            build_body(ctx, tc, nc, Qd[:], Cd[:], Wd[:], bd[:], Od[:])
    nc.finalize()
    return nc


_NC = None


def get_nc():
    global _NC
    if _NC is None:
        _NC = build_nc()
    return _NC


def kernel(Q, C, W, b):
    assert Q.shape == (B, QL, H) and C.shape == (B, D, H)
    nc = get_nc()
    in_maps = [
        {
            "Q": np.ascontiguousarray(Q[i], dtype=np.float32),
            "C": np.ascontiguousarray(C[i], dtype=np.float32),
            "W": np.ascontiguousarray(W, dtype=np.float32),
            "b": np.ascontiguousarray(b, dtype=np.float32),
        }
        for i in range(N_CORES)
    ]
    res = run_bass_kernel_spmd(nc, in_maps, core_ids=list(range(N_CORES)))
    return np.stack([res.results[i]["out"] for i in range(N_CORES)], axis=0)
